# revision 20
# baseline (speedup 1.0000x reference)
"""CoaT factorized-attention + CRPE block on 8 Trainium2 NeuronCores.

Sharding: pure data-parallel over batch B=32 -> 4 images per core.
Per-core layouts (all f16 matmul operands, fp32 PSUM accumulation):
  xT      [C=512, T=786]   feature-major input (host pre-transposes, f16)
  ek, v   [T, C]           token-major (GEMM-KV: lhsT=xT tiles, rhs=[Wk.T|Wv.T])
                           v tiles carry 8.0-valued ones columns so the kv
                           matmul also emits 8*softmax denominators (the 1/8
                           attention scale is folded into the reciprocal).
  q       [C, T]           feature-major (GEMM-Q: lhsT=Wq.T tiles, rhs=xT)
  v_feat  via PE transposes of token-major v (identity matmul) -> padded
          34x34 conv image buffer (borders zeroed once per ring buffer).
  kv      [c, d] per head-pair + denom cols, scaled on PSUM->SBUF copy.
  conv    taps split across PE (diag matmuls), DVE (fused stt), Pool (stt),
          first tap with bias on ScalarE.
  proj    out[T, 512] token-major (lhsT=attn feature-major, rhs=proj_w.T)

Software pipeline (engines execute in-order; the serial DVE tap chain of
image b must overlap PE GEMMs of image b+1):
  emit: s1(0); s2a(0); s1(1); s2b(0); s2a(1); s1(2); s2b(1); ...
  s1  = xt DMA, GEMM-KV, kv, GEMM-Q, v-transposes, vpad fill, vq copy
  s2a = conv chains + EV, factor-att + attn assembly
  s2b = proj + output DMA
"""

import numpy as np

import concourse.bass as bass
import concourse.bacc as bacc
import concourse.mybir as mybir
import concourse.tile as tile
from concourse.bass_utils import run_bass_kernel_spmd

F32 = mybir.dt.float32
F16 = mybir.dt.float16

NCORES = 8
B, N, C = 32, 785, 512
BL = B // NCORES            # 4 images per core
H = W = 28
HW = H * W                  # 784, N = 1 + HW
NH = 8                      # heads
CH = C // NH                # 64
SCALE = CH ** -0.5          # 1/8
PADW = 34                   # 28 + 2*3 (pad 3 covers 3x3/5x5/7x7 uniformly)
NP = 786                    # token columns padded even
NT = 7                      # token tiles: 6*128 + 17
TSIZES = [128, 128, 128, 128, 128, 128, 17]
TOFFS = [0, 128, 256, 384, 512, 640, 768]
VP = 132                    # per-head-pair pitch in v tiles (128 data + 2 ones + 2 pad)

# per channel-tile (=head-pair) conv tap grid (kernel size k); ct2 mixes
# h4(5x5)+h5(7x7) so it runs a 7x7 grid with the 5x5 weights zero-embedded.
CT_K = [3, 5, 7, 7]

# tap routing: per ct (n_pe, n_dve, n_sc). If n_pe == k*k the ct runs
# fully on PE (bias via ones-image matmul); otherwise ScalarE takes one
# even-offset tap (with the conv bias) and the rest are assigned
# PE-first, then DVE (ts+tt pair), then ScalarE-assisted (ScalarE does the
# tap multiply into tmp, DVE only the accumulate add).
CT_SPLIT = {0: (9, 0, 0), 1: (25, 0, 0), 2: (27, 9, 12), 3: (0, 24, 24)}
DVE_CHAIN = 14     # max serial add-chain length on DVE before forking


def _tap_base(k):
    return 3 - (k // 2)


def _plan_ct(ct):
    """Returns (full_pe, scalar_tap, pe_taps, dve_taps, sc_taps) as lists
    of tap indices ti = i*k+j into the ct's k x k grid."""
    k = CT_K[ct]
    n_pe, n_dve, n_sc = CT_SPLIT[ct]
    taps = list(range(k * k))
    if n_pe >= k * k:
        return True, None, taps, [], []
    base = _tap_base(k)
    scalar_tap = None
    for ti in taps:
        i, j = divmod(ti, k)
        if ((base + i) * PADW + base + j) % 2 == 0:
            scalar_tap = ti
            break
    rest = [t for t in taps if t != scalar_tap]
    assert n_pe + n_dve + n_sc == len(rest), (ct, n_pe, n_dve, n_sc, len(rest))
    return (False, scalar_tap, rest[:n_pe], rest[n_pe:n_pe + n_dve],
            rest[n_pe + n_dve:])


def _ct_needs_vq(ct):
    k = CT_K[ct]
    base = _tap_base(k)
    _, _, _, dve, sc = _plan_ct(ct)
    for ti in dve + sc:
        i, j = divmod(ti, k)
        if ((base + i) * PADW + base + j) % 2:
            return True
    return False


def build_conv_weights(w3, b3, w5, b5, w7, b7):
    """Per channel-tile tap weights [4][128, 49] and biases [128, 4]."""
    w3 = w3.reshape(128, 9).astype(np.float32)
    w5 = w5.reshape(192, 25).astype(np.float32)
    w7 = w7.reshape(192, 49).astype(np.float32)
    cw = np.zeros((4, 128, 49), np.float32)
    cw[0, :, :9] = w3
    cw[1, :, :25] = w5[:128]
    emb = np.zeros((64, 7, 7), np.float32)
    emb[:, 1:6, 1:6] = w5[128:192].reshape(64, 5, 5)
    cw[2, :64] = emb.reshape(64, 49)
    cw[2, 64:] = w7[:64]
    cw[3] = w7[64:192]
    cb = np.zeros((128, 4), np.float32)
    cb[:, 0] = b3
    cb[:, 1] = b5[:128]
    cb[:64, 2] = b5[128:192]
    cb[64:, 2] = b7[:64]
    cb[:, 3] = b7[64:192]
    return cw, cb


def _cwd_layout():
    entries = []
    for ct in range(4):
        full_pe, _, pe_taps, _, _ = _plan_ct(ct)
        if full_pe:
            entries.append((ct, -1))
        entries.extend((ct, ti) for ti in pe_taps)
    return entries


def build_nc(has_qkv_bias):
    nc = bacc.Bacc()

    def mm(out, lhsT, rhs, **kw):
        nc.tensor.matmul(out, lhsT, rhs, **kw)

    cwd_entries = _cwd_layout()
    n_diag = len(cwd_entries)

    xt_d = nc.dram_tensor("xt", [BL, C, NP], F16, kind="ExternalInput")
    wq_d = nc.dram_tensor("wqT", [C, C], F16, kind="ExternalInput")
    wkv_d = nc.dram_tensor("wkvT", [C, 2 * C], F16, kind="ExternalInput")
    pw_d = nc.dram_tensor("pwT", [C, C], F16, kind="ExternalInput")
    cw_d = nc.dram_tensor("cw", [4, 128, 49], F32, kind="ExternalInput")
    cb_d = nc.dram_tensor("cb", [128, 4], F32, kind="ExternalInput")
    id_d = nc.dram_tensor("ident", [128, 128], F16, kind="ExternalInput")
    if n_diag:
        cwd_d = nc.dram_tensor("cwd", [n_diag, 128, 128], F16, kind="ExternalInput")
    if has_qkv_bias:
        bq_d = nc.dram_tensor("bq", [128, 4], F32, kind="ExternalInput")
        bv_d = nc.dram_tensor("bv", [128, 4], F32, kind="ExternalInput")
        bkv_d = nc.dram_tensor("bkv", [1, 2 * C], F16, kind="ExternalInput")
    out_d = nc.dram_tensor("out", [BL, N, C], F32, kind="ExternalOutput")

    with tile.TileContext(nc) as tc:
        with (
            tc.tile_pool(name="const", bufs=1) as cpool,
            tc.tile_pool(name="xt", bufs=2) as xtpool,
            tc.tile_pool(name="ek", bufs=7) as ekpool,
            tc.tile_pool(name="vt", bufs=1) as vtpool,
            tc.tile_pool(name="qf", bufs=8) as qpool,
            tc.tile_pool(name="vp", bufs=1) as vppool,
            tc.tile_pool(name="vq", bufs=1) as vqpool,
            tc.tile_pool(name="ca", bufs=2) as capool,
            tc.tile_pool(name="ev", bufs=2) as evpool,
            tc.tile_pool(name="at", bufs=8) as atpool,
            tc.tile_pool(name="sm", bufs=8) as smpool,
            tc.tile_pool(name="ob", bufs=2) as obpool,
            tc.tile_pool(name="ps", bufs=4, space="PSUM") as pspool,
            tc.tile_pool(name="psc", bufs=2, space="PSUM") as pscpool,
        ):
            # ---- constants (loaded once) ----
            wq_t = [cpool.tile([128, C], F16, tag=f"wq{i}", name=f"wq{i}") for i in range(4)]
            wkv_t = [cpool.tile([128, 2 * C], F16, tag=f"wkv{i}", name=f"wkv{i}") for i in range(4)]
            pw_t = [cpool.tile([128, C], F16, tag=f"pw{i}", name=f"pw{i}") for i in range(4)]
            cw_t = [cpool.tile([128, 49], F32, tag=f"cw{i}", name=f"cw{i}") for i in range(4)]
            cb_t = cpool.tile([128, 4], F32, tag="cb")
            id_t = cpool.tile([128, 128], F16, tag="ident")
            for i in range(4):
                r = slice(128 * i, 128 * (i + 1))
                nc.sync.dma_start(wq_t[i][:], wq_d[r, :])
                nc.sync.dma_start(wkv_t[i][:], wkv_d[r, :])
                nc.sync.dma_start(pw_t[i][:], pw_d[r, :])
                nc.sync.dma_start(cw_t[i][:], cw_d[i])
            nc.sync.dma_start(cb_t[:], cb_d[:])
            nc.sync.dma_start(id_t[:], id_d[:])
            cwd_t = []
            for i in range(n_diag):
                dtl = cpool.tile([128, 128], F16, tag=f"cwd{i}", name=f"cwd{i}")
                nc.sync.dma_start(dtl[:], cwd_d[i])
                cwd_t.append(dtl)
            diag_idx = {e: i for i, e in enumerate(cwd_entries)}
            if any(_plan_ct(ct)[0] for ct in range(4)):
                onesimg = cpool.tile([128, HW], F16, tag="onesimg")
                nc.gpsimd.memset(onesimg[:], 1.0)
            if has_qkv_bias:
                bq_t = cpool.tile([128, 4], F32, tag="bq")
                bv_t = cpool.tile([128, 4], F32, tag="bv")
                bkv_t = cpool.tile([1, 2 * C], F16, tag="bkv")
                ones_t = cpool.tile([1, 128], F16, tag="ones")
                nc.sync.dma_start(bq_t[:], bq_d[:])
                nc.sync.dma_start(bv_t[:], bv_d[:])
                nc.sync.dma_start(bkv_t[:], bkv_d[:])
                nc.scalar.activation(ones_t[:], bkv_t[:, 0:128],
                    mybir.ActivationFunctionType.Identity, bias=1.0, scale=0.0)

            # ---- persistent tiles with one-time initialization ----
            # (persistent, not pooled: their constant regions — ones columns,
            # zero borders, zero off-diagonal blocks — survive across images)
            vt_p = []
            for i in range(7):
                vv = vtpool.tile([128, 4 * VP], F16, tag=f"vt{i}", name=f"vt{i}")
                nc.gpsimd.memset(vv[:], 0.0)
                nc.gpsimd.memset(
                    vv[:].rearrange("p (a b) -> p a b", a=4)[:, :, 128:130], 8.0)
                vt_p.append(vv)
            kv_p = []
            for i in range(8):
                kvt = smpool.tile([128, 128], F16, tag=f"kvsb{i}",
                                  name=f"kvsb{i}", bufs=1)
                nc.gpsimd.memset(kvt[:], 0.0)
                kv_p.append(kvt)
            vp_p, vq_p = {}, {}
            for ct in range(4):
                for i in range(2):
                    vp = vppool.tile([128, PADW, PADW], F16, tag=f"vp{ct}_{i}",
                                     name=f"vp{ct}_{i}")
                    nc.gpsimd.memset(vp[:], 0.0)
                    vp_p[(ct, i)] = vp
                    if _ct_needs_vq(ct):
                        vq = vqpool.tile([128, PADW, PADW], F16,
                                         tag=f"vq{ct}_{i}", name=f"vq{ct}_{i}")
                        nc.gpsimd.memset(vq[:], 0.0)
                        vq_p[(ct, i)] = vq

            # ---- per-image stage emitters ----
            state = {}

            def stage1(b):
                st = {}
                xt_t = []
                for kc in range(4):
                    t = xtpool.tile([128, NP], F16, tag=f"xt{kc}", name=f"xt{kc}")
                    nc.sync.dma_start(t[:], xt_d[b, 128 * kc:128 * (kc + 1), :])
                    xt_t.append(t)

                # GEMM-KV: token-major ek=exp(k) and v
                ek_t, v_t = [], []
                for tt in range(NT):
                    m = TSIZES[tt]
                    o = TOFFS[tt]
                    ek = ekpool.tile([128, C], F16, tag="ek")
                    vv = vt_p[tt]
                    for half in range(2):
                        ps = pspool.tile([128, 512], F32, tag="ps")
                        cols = slice(512 * half, 512 * (half + 1))
                        for kc in range(4):
                            mm(
                                ps[:m, :],
                                xt_t[kc][:, o:o + m],
                                wkv_t[kc][:, cols],
                                start=(kc == 0),
                                stop=(kc == 3 and not has_qkv_bias),
                            )
                        if has_qkv_bias:
                            mm(ps[:m, :], ones_t[:, :m], bkv_t[:, cols],
                               start=False, stop=True)
                        if half == 0:
                            nc.scalar.activation(
                                ek[:m, :], ps[:m, :],
                                mybir.ActivationFunctionType.Exp)
                        else:
                            nc.scalar.copy(
                                vv[:m].rearrange("p (a b) -> p a b", a=4)[:, :, 0:128],
                                ps[:m, :].rearrange("p (a b) -> p a b", a=4))
                    ek_t.append(ek)
                    v_t.append(vv)

                # kv per head-pair (+ scaled softmax denominators)
                kv_t = []
                for hp in range(4):
                    cs = slice(128 * hp, 128 * (hp + 1))
                    vs = slice(VP * hp, VP * hp + 130)
                    ps = pspool.tile([128, 512], F32, tag="ps")
                    for tt in range(NT):
                        m = TSIZES[tt]
                        mm(
                            ps[:, 0:130], ek_t[tt][:m, cs], v_t[tt][:m, vs],
                            start=(tt == 0), stop=(tt == NT - 1))
                    recip = smpool.tile([128, 1], F32, tag="recip")
                    nc.vector.reciprocal(recip[:], ps[:, 128:129])
                    kv = kv_p[hp + 4 * (b % 2)]
                    nc.scalar.activation(
                        kv[0:64, 0:64], ps[0:64, 0:64],
                        mybir.ActivationFunctionType.Copy,
                        scale=recip[0:64, :])
                    nc.scalar.activation(
                        kv[64:128, 64:128], ps[64:128, 64:128],
                        mybir.ActivationFunctionType.Copy,
                        scale=recip[64:128, :])
                    kv_t.append(kv)

                # GEMM-Q: feature-major q (f16)
                q_t = []
                for mo in range(4):
                    q = qpool.tile([128, NP], F16, tag="qf")
                    for cols in (slice(0, 512), slice(512, NP)):
                        w = cols.stop - cols.start
                        ps = pspool.tile([128, 512], F32, tag="ps")
                        for kc in range(4):
                            mm(
                                ps[:, 0:w],
                                wq_t[kc][:, 128 * mo:128 * (mo + 1)],
                                xt_t[kc][:, cols],
                                start=(kc == 0),
                                stop=(kc == 3),
                            )
                        if has_qkv_bias:
                            nc.scalar.activation(
                                q[:, cols], ps[:, 0:w],
                                mybir.ActivationFunctionType.Identity,
                                bias=bq_t[:, mo:mo + 1])
                        else:
                            nc.scalar.copy(q[:, cols], ps[:, 0:w])
                    q_t.append(q)

                # v feature-major via PE transposes -> padded image
                vpad_t, vq_t = [], {}
                for ct in range(4):
                    vs = pspool.tile([128, 512], F32, tag="ps")
                    vf = vs[:].bitcast(F16)  # [128, 1024] f16 view
                    for tt in range(NT):
                        m = TSIZES[tt]
                        o = TOFFS[tt]
                        nc.tensor.transpose(
                            vf[:, o:o + m],
                            v_t[tt][:m].rearrange(
                                "p (a b) -> p a b", a=4)[:, ct, 0:128],
                            id_t[:m, :m])
                    vp = vp_p[(ct, b % 2)]
                    if has_qkv_bias:
                        nc.scalar.activation(
                            vp[:, 3:31, 3:31],
                            vf[:, 1:N].rearrange("p (h w) -> p h w", h=H),
                            mybir.ActivationFunctionType.Identity,
                            bias=bv_t[:, ct:ct + 1])
                    else:
                        nc.scalar.copy(
                            vp[:, 3:31, 3:31],
                            vf[:, 1:N].rearrange("p (h w) -> p h w", h=H))
                    vpad_t.append(vp)
                    if _ct_needs_vq(ct):
                        vq = vq_p[(ct, b % 2)]
                        nc.vector.tensor_copy(
                            vq[:].rearrange("p a b -> p (a b)")[:, 0:1154],
                            vp[:].rearrange("p a b -> p (a b)")[:, 1:1155])
                        vq_t[ct] = vq

                st.update(kv_t=kv_t, q_t=q_t, vpad_t=vpad_t, vq_t=vq_t)
                return st

            def conv_ct(st, ct):
                """Conv chains + EV (into token-aligned evz) for one ct."""
                k = CT_K[ct]
                base = _tap_base(k)
                full_pe, scalar_tap, pe_taps, dve_taps, sc_taps = _plan_ct(ct)
                vpad_t, vq_t, q_t = st["vpad_t"], st["vq_t"], st["q_t"]

                def win(ti):
                    i, j = divmod(ti, k)
                    if ((base + i) * PADW + base + j) % 2:
                        return vq_t[ct][:, base + i:base + i + H,
                                        base + j - 1:base + j - 1 + W]
                    return vpad_t[ct][:, base + i:base + i + H,
                                      base + j:base + j + W]

                psc = None
                if pe_taps:
                    psc = pscpool.tile([128, 2, 512], F32, tag="pscv",
                                       name="pscv")
                    for hh in range(2):
                        yo = 14 * hh
                        first = True
                        if full_pe:
                            mm(psc[:, hh, 0:392],
                               cwd_t[diag_idx[(ct, -1)]][:],
                               onesimg[:, 392 * hh:392 * (hh + 1)],
                               start=True, stop=False)
                            first = False
                        for n_, ti in enumerate(pe_taps):
                            i, j = divmod(ti, k)
                            src = vpad_t[ct][:, base + i + yo:base + i + yo + 14,
                                             base + j:base + j + W]
                            mm(psc[:, hh, 0:392],
                               cwd_t[diag_idx[(ct, ti)]][:], src,
                               start=first, stop=(n_ == len(pe_taps) - 1))
                            first = False

                # non-PE taps: interleaved (mul_engine, ti), forked add-chains
                acc = None
                if scalar_tap is not None:
                    merged = []
                    si, di = 0, 0
                    # proportional interleave of sc and dve muls
                    tot_n = len(sc_taps) + len(dve_taps)
                    for n_ in range(tot_n):
                        take_sc = (si * max(len(dve_taps), 1)
                                   <= di * max(len(sc_taps), 1))
                        if si < len(sc_taps) and (take_sc or di >= len(dve_taps)):
                            merged.append(('sc', sc_taps[si])); si += 1
                        else:
                            merged.append(('dve', dve_taps[di])); di += 1

                    acc = capool.tile([128, H, W], F16, tag=f"ca{ct}",
                                      name=f"ca{ct}")
                    nc.scalar.activation(
                        acc[:], win(scalar_tap),
                        mybir.ActivationFunctionType.Identity,
                        bias=cb_t[:, ct:ct + 1],
                        scale=cw_t[ct][:, scalar_tap:scalar_tap + 1])
                    chains = [merged[i:i + DVE_CHAIN]
                              for i in range(0, len(merged), DVE_CHAIN)]
                    accs = [acc]
                    for cn, chain in enumerate(chains):
                        if cn == 0:
                            tgt = acc
                        else:
                            # seed a forked chain with a direct mul
                            eng, ti = chain[0]
                            tgt = capool.tile([128, H, W], F16,
                                              tag=f"cd{ct}_{cn}",
                                              name=f"cd{ct}_{cn}")
                            accs.append(tgt)
                            if eng == 'sc':
                                nc.scalar.activation(
                                    tgt[:], win(ti),
                                    mybir.ActivationFunctionType.Copy,
                                    scale=cw_t[ct][:, ti:ti + 1])
                            else:
                                nc.vector.tensor_scalar_mul(
                                    tgt[:], win(ti), cw_t[ct][:, ti:ti + 1])
                            chain = chain[1:]
                        for eng, ti in chain:
                            tmp = capool.tile([128, H, W], F16,
                                              tag=f"tp{ct}", name=f"tp{ct}",
                                              bufs=6)
                            if eng == 'sc':
                                nc.scalar.activation(
                                    tmp[:], win(ti),
                                    mybir.ActivationFunctionType.Copy,
                                    scale=cw_t[ct][:, ti:ti + 1])
                            else:
                                nc.vector.tensor_scalar_mul(
                                    tmp[:], win(ti), cw_t[ct][:, ti:ti + 1])
                            nc.vector.tensor_tensor(
                                tgt[:], tgt[:], tmp[:], op=mybir.AluOpType.add)
                    for extra in accs[1:]:
                        nc.vector.tensor_tensor(
                            acc[:], acc[:], extra[:], op=mybir.AluOpType.add)

                # EV = conv * q_img, written token-aligned into evz[:, 1:785]
                # (cols 0 and 785 zeroed so PE can add evz into fa PSUM)
                qimg = q_t[ct][:, 1:N].rearrange("p (h w) -> p h w", h=H)
                evz = evpool.tile([128, NP], F16, tag=f"ev{ct}",
                                  name=f"ev{ct}")
                nc.gpsimd.memset(evz[:, 0:1], 0.0)
                nc.gpsimd.memset(evz[:, N:NP], 0.0)
                evi = evz[:, 1:N].rearrange("p (h w) -> p h w", h=H)
                if psc is not None and acc is not None:
                    tot = capool.tile([128, H, W], F16, tag=f"tt{ct}",
                                      name=f"tt{ct}")
                    nc.vector.tensor_tensor(
                        tot[:].rearrange("p h w -> p (h w)").rearrange(
                            "p (a b) -> p a b", a=2),
                        psc[:, :, 0:392],
                        acc[:].rearrange("p h w -> p (h w)").rearrange(
                            "p (a b) -> p a b", a=2),
                        op=mybir.AluOpType.add)
                    nc.vector.tensor_tensor(
                        evi, tot[:], qimg, op=mybir.AluOpType.mult)
                elif psc is not None:
                    nc.vector.tensor_tensor(
                        evz[:, 1:N].rearrange("p (a b) -> p a b", a=2),
                        psc[:, :, 0:392],
                        q_t[ct][:, 1:N].rearrange("p (a b) -> p a b", a=2),
                        op=mybir.AluOpType.mult)
                else:
                    nc.vector.tensor_tensor(
                        evi, acc[:], qimg, op=mybir.AluOpType.mult)
                return evz

            def stage2a(b, st):
                # conv in ct order: EVs land early in the DVE queue so psc
                # ring slots recycle without waiting on the ct3 tap chain.
                evs = {}
                for ct in (0, 1, 2, 3):
                    evs[ct] = conv_ct(st, ct)

                # factor-att; crpe added on PE via identity-matmul accumulate
                attn_t = []
                for hp in range(4):
                    at = atpool.tile([128, N], F16, tag="attn")
                    evz = evs[hp]
                    for hh, cols in enumerate((slice(0, 512), slice(512, NP))):
                        w = cols.stop - cols.start
                        ps = pspool.tile([128, 512], F32, tag="ps")
                        mm(ps[:, 0:w], st["kv_t"][hp][:],
                           st["q_t"][hp][:, cols], start=True, stop=False)
                        mm(ps[:, 0:w], id_t[:], evz[:, cols],
                           start=False, stop=True)
                        if hh == 0:
                            nc.scalar.copy(at[:, 0:512], ps[:, 0:512])
                        else:
                            nc.scalar.copy(at[:, 512:N], ps[:, 0:273])
                    attn_t.append(at)
                st["attn_t"] = attn_t

            def stage2b(b, st):
                for tt in range(NT):
                    m = TSIZES[tt]
                    o = TOFFS[tt]
                    ps = pspool.tile([128, 512], F32, tag="ps")
                    for kc in range(4):
                        mm(
                            ps[:m, :], st["attn_t"][kc][:, o:o + m],
                            pw_t[kc][:], start=(kc == 0), stop=(kc == 3))
                    ob = obpool.tile([128, C], F32, tag="ob")
                    nc.scalar.copy(ob[:m, :], ps[:m, :])
                    nc.sync.dma_start(out_d[b, o:o + m, :], ob[:m, :])

            # ---- pipelined emission ----
            # s1(0); s2a(0); s1(1); s2b(0); s2a(1); s1(2); s2b(1); ...
            state[0] = stage1(0)
            stage2a(0, state[0])
            for b in range(1, BL):
                state[b] = stage1(b)
                stage2b(b - 1, state[b - 1])
                stage2a(b, state[b])
            stage2b(BL - 1, state[BL - 1])

    nc.compile()
    return nc


_NC_CACHE = {}


def _get_nc(has_qkv_bias):
    key = (bool(has_qkv_bias), DVE_CHAIN,
           tuple(sorted(CT_SPLIT.items())))
    if key not in _NC_CACHE:
        _NC_CACHE[key] = build_nc(has_qkv_bias)
    return _NC_CACHE[key]


def build_host_inputs(x, qkv_w, qkv_b, proj_w, w3, b3, w5, b5, w7, b7):
    """Shared + per-core input maps (host-side prep)."""
    wqT = np.ascontiguousarray(qkv_w[0:C].T).astype(np.float16)
    wkvT = np.ascontiguousarray(np.concatenate(
        [qkv_w[C:2 * C].T, qkv_w[2 * C:3 * C].T], axis=1)).astype(np.float16)
    pwT = np.ascontiguousarray(proj_w.T).astype(np.float16)
    cw, cb = build_conv_weights(
        np.asarray(w3, np.float32), np.asarray(b3, np.float32),
        np.asarray(w5, np.float32), np.asarray(b5, np.float32),
        np.asarray(w7, np.float32), np.asarray(b7, np.float32))

    ar = np.arange(128)
    diags = []
    for ct, ti in _cwd_layout():
        d = np.zeros((128, 128), np.float16)
        d[ar, ar] = (cb[:, ct] if ti < 0 else cw[ct][:, ti]).astype(np.float16)
        diags.append(d)

    shared = {
        "wqT": wqT, "wkvT": wkvT, "pwT": pwT,
        "cw": cw, "cb": cb,
        "ident": np.eye(128, dtype=np.float16),
    }
    if diags:
        shared["cwd"] = np.stack(diags)
    has_bias = bool(np.any(qkv_b))
    if has_bias:
        shared["bq"] = np.ascontiguousarray(qkv_b[0:C].reshape(4, 128).T)
        shared["bv"] = np.ascontiguousarray(qkv_b[2 * C:3 * C].reshape(4, 128).T)
        shared["bkv"] = np.ascontiguousarray(
            qkv_b[C:3 * C].reshape(1, 2 * C)).astype(np.float16)

    in_maps = []
    for core in range(NCORES):
        xs = x[core * BL:(core + 1) * BL]            # [4, 785, 512]
        xt = np.zeros((BL, C, NP), np.float16)
        xt[:, :, :N] = xs.transpose(0, 2, 1)
        m = {"xt": xt}
        m.update(shared)
        in_maps.append(m)
    return shared, in_maps, has_bias


def kernel(x, qkv_w, qkv_b, proj_w, proj_b, w3, b3, w5, b5, w7, b7, H=28, W=28):
    x = np.asarray(x, np.float32)
    qkv_w = np.asarray(qkv_w, np.float32)
    qkv_b = np.asarray(qkv_b, np.float32)
    proj_w = np.asarray(proj_w, np.float32)
    proj_b = np.asarray(proj_b, np.float32)
    assert x.shape == (B, N, C), x.shape
    assert int(H) == 28 and int(W) == 28

    _, in_maps, has_bias = build_host_inputs(
        x, qkv_w, qkv_b, proj_w, w3, b3, w5, b5, w7, b7)
    nc = _get_nc(has_bias)

    res = run_bass_kernel_spmd(nc, in_maps, list(range(NCORES)))
    global LAST_RESULT
    LAST_RESULT = res
    out = np.concatenate([r["out"] for r in res.results], axis=0)
    out = out + proj_b[None, None, :]
    return out.astype(np.float32)
